# revision 12
# baseline (speedup 1.0000x reference)
"""GPTQ 4-bit quantized linear: out = x @ dequant(qweight, qzeros, scales, g_idx) + bias.

Full shapes: x [8192, 4096] fp16, qweight [512, 4096] int32 (8x 4-bit packed
along K), qzeros [32, 512] int32, scales [32, 4096] fp16, g_idx [4096] int32
(k // 128), bias [4096] fp16.  Output [8192, 4096] fp16.

Strategy: 2 (M) x 4 (N) grid over 8 NeuronCores.  Per core: M=4096, N=1024,
K=4096, all in fp16 on the PE -- but via one level of Strassen, which cuts
PE cycles to 7/8 of the direct matmul (the PE at 78.6 TF/s fp16 is the
bottleneck; fp8 DoubleRow was measured to give 2x FLOPs per cycle but the
3-term error-corrected decomposition it needs costs 3x FLOPs, a net loss).

Per core split M=2x2048, K=2x2048, N=2x512:
  P1=(A11+A22)(B11+B22)  P2=(A21+A22)B11  P3=A11(B12-B22)  P4=A22(B21-B11)
  P5=(A11+A12)B22        P6=(A21-A11)(B11+B12)  P7=(A12-A22)(B21+B22)
  C11=P1+P4-P5+P7  C12=P3+P5  C21=P2+P4  C22=P1-P2+P3+P6

The host dequantizes W and precomputes the 7 fp16 A- and B-combos (adds of
quadrants; psum stays fp32 so the fp16 combo rounding is benign -- measured
rel err ~9e-4).  The device, per 128-row m'-block, accumulates the seven
P-products in seven PSUM banks (16 matmuls of [128k,128m]x[128k,512n] each)
and combines them with 12 DVE ops ordered so PSUM banks free early.  B-combos
(14.7MB fp16) stay resident in SBUF; A-combos stream per block on SyncE.
"""

import os
import sys

import numpy as np

for _p in ("/opt/trn_rl_repo",):
    if _p not in sys.path and os.path.isdir(_p):
        sys.path.insert(0, _p)

import concourse.bass as bass
import concourse.mybir as mybir
import concourse.tile as tile
from concourse import bacc
from concourse.bass_utils import run_bass_kernel_spmd

dt = mybir.dt

P = 128          # partitions
JP = 8           # 4-bit values per int32
GROUP = 128      # quant group size
NPS = 512        # psum free width / n'-quadrant width
NPROD = 7


def build_program(K, M, N):
    """One-core SPMD program: Strassen 1-level over [M=4096,K=4096,N=1024]."""
    KH, MH, NH = K // 2, M // 2, N // 2
    KT = KH // P         # 16 k'-tiles per product
    MB = MH // P         # 16 m'-blocks
    assert NH == NPS

    nc = bacc.Bacc("TRN2", target_bir_lowering=False)

    ac = nc.dram_tensor("ac", [NPROD, MB, P, KT, P], dt.float16, kind="ExternalInput")
    bc = nc.dram_tensor("bc", [NPROD, P, KT, NPS], dt.float16, kind="ExternalInput")
    bs = nc.dram_tensor("bs", [P, N], dt.float16, kind="ExternalInput")
    out = nc.dram_tensor("out", [M, N], dt.float16, kind="ExternalOutput")

    add = mybir.AluOpType.add
    sub = mybir.AluOpType.subtract

    from contextlib import ExitStack

    with tile.TileContext(nc) as tc, ExitStack() as ctx:
        const = ctx.enter_context(tc.tile_pool(name="const", bufs=1))
        bpool = ctx.enter_context(tc.tile_pool(name="bpool", bufs=1))
        apool = ctx.enter_context(tc.tile_pool(name="apool", bufs=2))
        tpool = ctx.enter_context(tc.tile_pool(name="tpool", bufs=8))
        opool = ctx.enter_context(tc.tile_pool(name="opool", bufs=4))
        psum = ctx.enter_context(tc.tile_pool(name="psum", bufs=8, space="PSUM"))

        # PE warmup: dummy fp16 matmuls with no DMA dependency so the HAM
        # clock-gate opens (1.2 -> 2.4 GHz) before the first real matmul.
        warm_src = const.tile([P, NPS], dt.float16)
        nc.gpsimd.memset(warm_src[:], 0.0)
        warm_ps = psum.tile([P, NPS], dt.float32, tag="ps")
        NWARM = 12
        for wi in range(NWARM):
            nc.tensor.matmul(
                warm_ps[:], warm_src[:, :P], warm_src[:],
                start=(wi == 0), stop=(wi == NWARM - 1),
            )

        bias_t = const.tile([P, N], dt.float16)
        nc.sync.dma_start(bias_t[:], bs[:])

        # resident B-combo tiles: [128, 16, 512] fp16 per product (16KB
        # lines).  Each product's B is fetched as two half-tiles on the two
        # spare rings (gpsimd + ACT) in product order, so B[prod] arrival
        # tracks the product-major phase-0 consumption point.
        KTH = KT // 2
        BT = []
        for prod in range(NPROD):
            t = bpool.tile([P, KT, NPS], dt.float16, name=f"b{prod}")
            nc.gpsimd.dma_start(t[:, 0:KTH, :], bc[prod, :, 0:KTH, :])
            nc.scalar.dma_start(t[:, KTH:KT, :], bc[prod, :, KTH:KT, :])
            BT.append(t)

        def a_load(prod, mb, eng):
            t = apool.tile([P, KT, P], dt.float16, tag=f"a{prod}",
                           name=f"a_t{prod}")
            eng.dma_start(t[:], ac[prod, mb])
            return t

        def run_product(prod, at, name):
            ps = psum.tile([P, NPS], dt.float32, tag="ps", name=name)
            for kt in range(KT):
                nc.tensor.matmul(
                    ps[:], at[:, kt, :], BT[prod][:, kt, :],
                    start=(kt == 0), stop=(kt == KT - 1),
                )
            return ps

        def combine_and_store(mb, ss, fused_psums=None):
            """ss: 7 P-tiles in SBUF (entries 5/6 None with fused_psums =
            (P6ps, P7ps) psum handles read directly -- DVE takes at most one
            PSUM operand per op)."""
            S1, S2, S3, S4, S5, S6, S7 = ss
            P6ps, P7ps = fused_psums if fused_psums else (None, None)

            def tt(name, a, b, op):
                t = tpool.tile([P, NPS], dt.float16, tag="u", name=name, bufs=6)
                nc.vector.tensor_tensor(t[:], a[:], b[:], op=op)
                return t

            u1 = tt("u1", S1, S4, add)
            u2 = (tt("u2", P7ps, S5, sub) if S7 is None
                  else tt("u2", S7, S5, sub))
            u3 = tt("u3", u1, u2, add)             # C11 pre-bias
            u4 = tt("u4", S3, S5, add)             # C12 pre-bias
            u5 = tt("u5", S2, S4, add)             # C21 pre-bias
            u6 = tt("u6", S1, S2, sub)
            u7 = (tt("u7", P6ps, S3, add) if S6 is None
                  else tt("u7", S6, S3, add))
            u8 = tt("u8", u6, u7, add)             # C22 pre-bias

            m1 = mb * P
            m2 = MH + mb * P
            for name, u, nsl, mrow, st_eng in (
                ("o11", u3, slice(0, NPS), m1, nc.sync),
                ("o12", u4, slice(NPS, N), m1, nc.scalar),
                ("o21", u5, slice(0, NPS), m2, nc.sync),
                ("o22", u8, slice(NPS, N), m2, nc.scalar),
            ):
                o = opool.tile([P, NPS], dt.float16, tag="o", name=name)
                nc.vector.tensor_tensor(o[:], u[:], bias_t[:, nsl], op=add)
                st_eng.dma_start(out[mrow:mrow + P, nsl], o[:])

        # ---- phase 0: blocks 0..1 product-major, so compute starts as soon
        # as B[0] lands and trails the two B rings product by product.  Every
        # psum drains to SBUF right after its 16 matmuls (DVE/ACT split).
        PH0 = 3
        ph0_s = [[None] * NPROD for _ in range(PH0)]
        for prod in range(NPROD):
            ats = [a_load(prod, mb, nc.sync) for mb in range(PH0)]
            for mb in range(PH0):
                ps = run_product(prod, ats[mb], f"ps_p{prod}_{mb}")
                s = tpool.tile([P, NPS], dt.float16, tag="s",
                               name=f"s0_{prod}_{mb}", bufs=22)
                if prod % 2 == 0:
                    nc.vector.tensor_copy(s[:], ps[:])
                else:
                    nc.scalar.activation(
                        s[:], ps[:], mybir.ActivationFunctionType.Copy
                    )
                ph0_s[mb][prod] = s
        for mb in range(PH0):
            combine_and_store(mb, ph0_s[mb])

        # ---- phase 1: remaining blocks, block-major (B resident by now)
        A_ENG = [nc.sync, nc.sync, nc.sync, nc.sync, nc.gpsimd, nc.gpsimd,
                 nc.gpsimd]
        for mb in range(PH0, MB):
            AT = [a_load(prod, mb, A_ENG[prod]) for prod in range(NPROD)]
            pss = [run_product(prod, AT[prod], f"ps{prod}")
                   for prod in range(NPROD)]

            # P1..P5 are read twice -> SBUF copies (DVE/ACT split); P6/P7
            # are read once and fused as direct psum operands.
            ss = []
            for prod in range(5):
                s = tpool.tile([P, NPS], dt.float16, tag="s", name=f"s{prod}", bufs=22)
                if prod in (1, 3):
                    nc.scalar.activation(
                        s[:], pss[prod][:], mybir.ActivationFunctionType.Copy
                    )
                else:
                    nc.vector.tensor_copy(s[:], pss[prod][:])
                ss.append(s)
            combine_and_store(mb, ss + [None, None],
                              fused_psums=(pss[5], pss[6]))
    nc.finalize()
    return nc


def _pack_a(combo):
    """combo: [MH, KH] fp16 -> [MB, 128, KT, 128] with m=128*mb+mm, k=128*kt+p."""
    MH, KH = combo.shape
    a = combo.reshape(MH // P, P, KH // P, P)            # (mb, mm, kt, p)
    return np.ascontiguousarray(a.transpose(0, 3, 2, 1))  # (mb, p, kt, mm)


def _pack_b(combo):
    """combo: [KH, NPS] fp16 -> [128, KT, NPS] with k=128*kt+p."""
    KH = combo.shape[0]
    a = combo.reshape(KH // P, P, NPS)                   # (kt, p, n)
    return np.ascontiguousarray(a.transpose(1, 0, 2))    # (p, kt, n)


def host_prep(x, qweight, qzeros, scales, g_idx, bias, m_split, n_split):
    """Dequantize W, build fp16 Strassen A/B combos, pack per core."""
    M_full, K = x.shape
    G, N_full = scales.shape
    Mc = M_full // m_split
    Nc = N_full // n_split
    KH, MHc, NHc = K // 2, Mc // 2, Nc // 2

    shifts = (np.arange(JP, dtype=np.int32) * 4)
    w = ((qweight[:, None, :] >> shifts[None, :, None]) & 15).reshape(K, N_full)
    z = ((qzeros[:, :, None] >> shifts[None, None, :]) & 15).reshape(G, N_full) + 1
    cg = np.asarray(g_idx[::GROUP])
    assert np.array_equal(np.repeat(cg, GROUP), np.asarray(g_idx)), \
        "g_idx must be uniform within 128-wide k chunks"
    iw = (w - z[cg].repeat(GROUP, axis=0)).astype(np.float32)
    W16 = (iw * np.asarray(scales, np.float32)[cg].repeat(GROUP, axis=0)
           ).astype(np.float16)

    x = np.asarray(x)
    bias = np.asarray(bias)

    a_shards = []
    for mi in range(m_split):
        xm = x[mi * Mc:(mi + 1) * Mc]
        A11 = xm[:MHc, :KH]; A12 = xm[:MHc, KH:]
        A21 = xm[MHc:, :KH]; A22 = xm[MHc:, KH:]
        combos = (A11 + A22, A21 + A22, A11, A22,
                  A11 + A12, A21 - A11, A12 - A22)
        arr = np.empty((NPROD, MHc // P, P, KH // P, P), np.float16)
        for i, c in enumerate(combos):
            arr[i] = _pack_a(np.ascontiguousarray(c))
        a_shards.append(arr)

    b_shards = []
    for ni in range(n_split):
        Wc = W16[:, ni * Nc:(ni + 1) * Nc]
        B11 = Wc[:KH, :NHc]; B12 = Wc[:KH, NHc:]
        B21 = Wc[KH:, :NHc]; B22 = Wc[KH:, NHc:]
        combos = (B11 + B22, B11, B12 - B22, B21 - B11,
                  B22, B11 + B12, B21 + B22)
        arr = np.empty((NPROD, P, KH // P, NHc), np.float16)
        for i, c in enumerate(combos):
            arr[i] = _pack_b(np.ascontiguousarray(c))
        b_shards.append(arr)

    in_maps = []
    for mi in range(m_split):
        for ni in range(n_split):
            in_maps.append({
                "ac": a_shards[mi],
                "bc": b_shards[ni],
                "bs": np.ascontiguousarray(
                    np.broadcast_to(bias[ni * Nc:(ni + 1) * Nc], (P, Nc))
                ),
            })
    return in_maps, Mc, Nc


_PROGRAM_CACHE = {}


def _get_program(K, M, N):
    key = (K, M, N)
    if key not in _PROGRAM_CACHE:
        _PROGRAM_CACHE[key] = build_program(K, M, N)
    return _PROGRAM_CACHE[key]


def kernel(x, qweight, qzeros, scales, g_idx, bias, trace=False, trace_kwargs=None):
    m_split, n_split = 2, 4
    x = np.asarray(x)
    qweight = np.asarray(qweight)
    qzeros = np.asarray(qzeros)
    scales = np.asarray(scales)
    g_idx = np.asarray(g_idx)
    bias = np.asarray(bias)
    M_full, K = x.shape
    N_full = scales.shape[1]
    in_maps, M, N = host_prep(x, qweight, qzeros, scales, g_idx, bias,
                              m_split, n_split)
    nc = _get_program(K, M, N)
    kw = {}
    if trace:
        kw = dict(trace=True, **(trace_kwargs or {}))
    rb = run_bass_kernel_spmd(nc, in_maps, list(range(m_split * n_split)), **kw)
    out = np.empty((M_full, N_full), dtype=np.float16)
    ci = 0
    for mi in range(m_split):
        for ni in range(n_split):
            out[mi * M:(mi + 1) * M, ni * N:(ni + 1) * N] = rb.results[ci]["out"]
            ci += 1
    kernel.last_results = rb
    return out


# revision 13
# speedup vs baseline: 1.1011x; 1.1011x over previous
"""GPTQ 4-bit quantized linear: out = x @ dequant(qweight, qzeros, scales, g_idx) + bias.

Full shapes: x [8192, 4096] fp16, qweight [512, 4096] int32 (8x 4-bit packed
along K), qzeros [32, 512] int32, scales [32, 4096] fp16, g_idx [4096] int32
(k // 128), bias [4096] fp16.  Output [8192, 4096] fp16.

Strategy: 2 (M) x 4 (N) grid over 8 NeuronCores.  Per core: M=4096, N=1024,
K=4096, all in fp16 on the PE -- but via one level of Strassen, which cuts
PE cycles to 7/8 of the direct matmul (the PE at 78.6 TF/s fp16 is the
bottleneck; fp8 DoubleRow was measured to give 2x FLOPs per cycle but the
3-term error-corrected decomposition it needs costs 3x FLOPs, a net loss).

Per core split M=2x2048, K=2x2048, N=2x512:
  P1=(A11+A22)(B11+B22)  P2=(A21+A22)B11  P3=A11(B12-B22)  P4=A22(B21-B11)
  P5=(A11+A12)B22        P6=(A21-A11)(B11+B12)  P7=(A12-A22)(B21+B22)
  C11=P1+P4-P5+P7  C12=P3+P5  C21=P2+P4  C22=P1-P2+P3+P6

The host dequantizes W and precomputes the 7 fp16 A- and B-combos (adds of
quadrants; psum stays fp32 so the fp16 combo rounding is benign -- measured
rel err ~9e-4).  The device, per 128-row m'-block, accumulates the seven
P-products in seven PSUM banks (16 matmuls of [128k,128m]x[128k,512n] each)
and combines them with 12 DVE ops ordered so PSUM banks free early.  B-combos
(14.7MB fp16) stay resident in SBUF; A-combos stream per block on SyncE.
"""

import os
import sys

import numpy as np

for _p in ("/opt/trn_rl_repo",):
    if _p not in sys.path and os.path.isdir(_p):
        sys.path.insert(0, _p)

import concourse.bass as bass
import concourse.mybir as mybir
import concourse.tile as tile
from concourse import bacc
from concourse.bass_utils import run_bass_kernel_spmd

dt = mybir.dt

P = 128          # partitions
JP = 8           # 4-bit values per int32
GROUP = 128      # quant group size
NPS = 512        # psum free width / n'-quadrant width
NPROD = 7


def build_program(K, M, N):
    """One-core SPMD program: Strassen 1-level over [M=4096,K=4096,N=1024]."""
    KH, MH, NH = K // 2, M // 2, N // 2
    KT = KH // P         # 16 k'-tiles per product
    MB = MH // P         # 16 m'-blocks
    assert NH == NPS

    nc = bacc.Bacc("TRN2", target_bir_lowering=False)

    ac = nc.dram_tensor("ac", [NPROD, MB, P, KT, P], dt.float16, kind="ExternalInput")
    bc = nc.dram_tensor("bc", [NPROD, P, KT, NPS], dt.float16, kind="ExternalInput")
    bs = nc.dram_tensor("bs", [P, N], dt.float16, kind="ExternalInput")
    out = nc.dram_tensor("out", [M, N], dt.float16, kind="ExternalOutput")

    add = mybir.AluOpType.add
    sub = mybir.AluOpType.subtract

    from contextlib import ExitStack

    with tile.TileContext(nc) as tc, ExitStack() as ctx:
        const = ctx.enter_context(tc.tile_pool(name="const", bufs=1))
        bpool = ctx.enter_context(tc.tile_pool(name="bpool", bufs=2))
        apool = ctx.enter_context(tc.tile_pool(name="apool", bufs=12))
        cpool = ctx.enter_context(tc.tile_pool(name="cpool", bufs=18))
        opool = ctx.enter_context(tc.tile_pool(name="opool", bufs=6))
        psum = ctx.enter_context(tc.tile_pool(name="psum", bufs=8, space="PSUM"))

        # PE warmup: dummy fp16 matmuls with no DMA dependency so the HAM
        # clock-gate opens (1.2 -> 2.4 GHz) before the first real matmul.
        warm_src = const.tile([P, NPS], dt.float16)
        nc.gpsimd.memset(warm_src[:], 0.0)
        warm_ps = psum.tile([P, NPS], dt.float32, tag="ps")
        NWARM = 12
        for wi in range(NWARM):
            nc.tensor.matmul(
                warm_ps[:], warm_src[:, :P], warm_src[:],
                start=(wi == 0), stop=(wi == NWARM - 1),
            )

        bias_t = const.tile([P, N], dt.float16)
        nc.sync.dma_start(bias_t[:], bs[:])
        b0 = bias_t[:, 0:NPS]
        b1 = bias_t[:, NPS:N]

        # Global product-major sweep: for each product p, stream its A tiles
        # across all 16 m'-blocks against the single resident B[p] tile
        # (16KB/partition, double-buffered).  Startup only waits for
        # B[0]+A[0,0] (~2.6MB); B prefetch needs just 38GB/s.  Per-block
        # drains maintain 4 running fp16 accumulators via tensor_tensor ops
        # with ONE psum operand each; subtractions are ordered so the sign
        # works out with no negation pass, and bias rides the init op:
        #   C11 = P7-(P5-(P1+b0+P4));  C12 = (P3+b1)+P5;  C21 = (P2+b0)+P4
        #   C22 = P6+(P3-(P2-(P1+b1)))
        KTH = KT // 2
        a11 = [None] * MB; a12 = [None] * MB
        a21 = [None] * MB; a22 = [None] * MB

        def acc(tag, mb, in0, in1, op):
            t = cpool.tile([P, NPS], dt.float16, tag=tag,
                           name=f"{tag}_{mb}", bufs=18)
            nc.vector.tensor_tensor(t[:], in0, in1, op=op)
            return t

        def store(mb, u, nsl, upper):
            mrow = (mb * P) if upper else (MH + mb * P)
            nc.scalar.dma_start(out[mrow:mrow + P, nsl], u[:])

        for prod in range(NPROD):
            bt = bpool.tile([P, KT, NPS], dt.float16, tag="b", name=f"b{prod}")
            nc.gpsimd.dma_start(bt[:, 0:KTH, :], bc[prod, :, 0:KTH, :])
            nc.scalar.dma_start(bt[:, KTH:KT, :], bc[prod, :, KTH:KT, :])
            for mb in range(MB):
                at = apool.tile([P, KT, P], dt.float16, tag="a",
                                name=f"a_t{prod}_{mb}", bufs=12)
                eng = nc.sync if mb % 2 == 0 else nc.gpsimd
                eng.dma_start(at[:], ac[prod, mb])
                ps = psum.tile([P, NPS], dt.float32, tag="ps",
                               name=f"ps{prod}_{mb}")
                for kt in range(KT):
                    nc.tensor.matmul(
                        ps[:], at[:, kt, :], bt[:, kt, :],
                        start=(kt == 0), stop=(kt == KT - 1),
                    )
                add = mybir.AluOpType.add
                sub = mybir.AluOpType.subtract
                if prod == 0:      # P1
                    a11[mb] = acc("c11", mb, ps[:], b0, add)
                    a22[mb] = acc("c22", mb, ps[:], b1, add)
                elif prod == 1:    # P2
                    a21[mb] = acc("c21", mb, ps[:], b0, add)
                    a22[mb] = acc("c22", mb, ps[:], a22[mb][:], sub)
                elif prod == 2:    # P3
                    a12[mb] = acc("c12", mb, ps[:], b1, add)
                    a22[mb] = acc("c22", mb, ps[:], a22[mb][:], sub)
                elif prod == 3:    # P4
                    a11[mb] = acc("c11", mb, ps[:], a11[mb][:], add)
                    o21 = opool.tile([P, NPS], dt.float16, tag="o",
                                     name="o21")
                    nc.vector.tensor_tensor(o21[:], ps[:], a21[mb][:], op=add)
                    store(mb, o21, slice(0, NPS), False)
                elif prod == 4:    # P5
                    a11[mb] = acc("c11", mb, ps[:], a11[mb][:], sub)
                    o12 = opool.tile([P, NPS], dt.float16, tag="o",
                                     name="o12")
                    nc.vector.tensor_tensor(o12[:], ps[:], a12[mb][:], op=add)
                    store(mb, o12, slice(NPS, N), True)
                elif prod == 5:    # P6
                    o22 = opool.tile([P, NPS], dt.float16, tag="o",
                                     name="o22")
                    nc.vector.tensor_tensor(o22[:], ps[:], a22[mb][:], op=add)
                    store(mb, o22, slice(NPS, N), False)
                else:              # P7
                    o11 = opool.tile([P, NPS], dt.float16, tag="o",
                                     name="o11")
                    nc.vector.tensor_tensor(o11[:], ps[:], a11[mb][:], op=sub)
                    store(mb, o11, slice(0, NPS), True)
    nc.finalize()
    return nc


def _pack_a(combo):
    """combo: [MH, KH] fp16 -> [MB, 128, KT, 128] with m=128*mb+mm, k=128*kt+p."""
    MH, KH = combo.shape
    a = combo.reshape(MH // P, P, KH // P, P)            # (mb, mm, kt, p)
    return np.ascontiguousarray(a.transpose(0, 3, 2, 1))  # (mb, p, kt, mm)


def _pack_b(combo):
    """combo: [KH, NPS] fp16 -> [128, KT, NPS] with k=128*kt+p."""
    KH = combo.shape[0]
    a = combo.reshape(KH // P, P, NPS)                   # (kt, p, n)
    return np.ascontiguousarray(a.transpose(1, 0, 2))    # (p, kt, n)


def host_prep(x, qweight, qzeros, scales, g_idx, bias, m_split, n_split):
    """Dequantize W, build fp16 Strassen A/B combos, pack per core."""
    M_full, K = x.shape
    G, N_full = scales.shape
    Mc = M_full // m_split
    Nc = N_full // n_split
    KH, MHc, NHc = K // 2, Mc // 2, Nc // 2

    shifts = (np.arange(JP, dtype=np.int32) * 4)
    w = ((qweight[:, None, :] >> shifts[None, :, None]) & 15).reshape(K, N_full)
    z = ((qzeros[:, :, None] >> shifts[None, None, :]) & 15).reshape(G, N_full) + 1
    cg = np.asarray(g_idx[::GROUP])
    assert np.array_equal(np.repeat(cg, GROUP), np.asarray(g_idx)), \
        "g_idx must be uniform within 128-wide k chunks"
    iw = (w - z[cg].repeat(GROUP, axis=0)).astype(np.float32)
    W16 = (iw * np.asarray(scales, np.float32)[cg].repeat(GROUP, axis=0)
           ).astype(np.float16)

    x = np.asarray(x)
    bias = np.asarray(bias)

    a_shards = []
    for mi in range(m_split):
        xm = x[mi * Mc:(mi + 1) * Mc]
        A11 = xm[:MHc, :KH]; A12 = xm[:MHc, KH:]
        A21 = xm[MHc:, :KH]; A22 = xm[MHc:, KH:]
        combos = (A11 + A22, A21 + A22, A11, A22,
                  A11 + A12, A21 - A11, A12 - A22)
        arr = np.empty((NPROD, MHc // P, P, KH // P, P), np.float16)
        for i, c in enumerate(combos):
            arr[i] = _pack_a(np.ascontiguousarray(c))
        a_shards.append(arr)

    b_shards = []
    for ni in range(n_split):
        Wc = W16[:, ni * Nc:(ni + 1) * Nc]
        B11 = Wc[:KH, :NHc]; B12 = Wc[:KH, NHc:]
        B21 = Wc[KH:, :NHc]; B22 = Wc[KH:, NHc:]
        combos = (B11 + B22, B11, B12 - B22, B21 - B11,
                  B22, B11 + B12, B21 + B22)
        arr = np.empty((NPROD, P, KH // P, NHc), np.float16)
        for i, c in enumerate(combos):
            arr[i] = _pack_b(np.ascontiguousarray(c))
        b_shards.append(arr)

    in_maps = []
    for mi in range(m_split):
        for ni in range(n_split):
            in_maps.append({
                "ac": a_shards[mi],
                "bc": b_shards[ni],
                "bs": np.ascontiguousarray(
                    np.broadcast_to(bias[ni * Nc:(ni + 1) * Nc], (P, Nc))
                ),
            })
    return in_maps, Mc, Nc


_PROGRAM_CACHE = {}


def _get_program(K, M, N):
    key = (K, M, N)
    if key not in _PROGRAM_CACHE:
        _PROGRAM_CACHE[key] = build_program(K, M, N)
    return _PROGRAM_CACHE[key]


def kernel(x, qweight, qzeros, scales, g_idx, bias, trace=False, trace_kwargs=None):
    m_split, n_split = 2, 4
    x = np.asarray(x)
    qweight = np.asarray(qweight)
    qzeros = np.asarray(qzeros)
    scales = np.asarray(scales)
    g_idx = np.asarray(g_idx)
    bias = np.asarray(bias)
    M_full, K = x.shape
    N_full = scales.shape[1]
    in_maps, M, N = host_prep(x, qweight, qzeros, scales, g_idx, bias,
                              m_split, n_split)
    nc = _get_program(K, M, N)
    kw = {}
    if trace:
        kw = dict(trace=True, **(trace_kwargs or {}))
    rb = run_bass_kernel_spmd(nc, in_maps, list(range(m_split * n_split)), **kw)
    out = np.empty((M_full, N_full), dtype=np.float16)
    ci = 0
    for mi in range(m_split):
        for ni in range(n_split):
            out[mi * M:(mi + 1) * M, ni * N:(ni + 1) * N] = rb.results[ci]["out"]
            ci += 1
    kernel.last_results = rb
    return out


# revision 14
# speedup vs baseline: 1.1048x; 1.0033x over previous
"""GPTQ 4-bit quantized linear: out = x @ dequant(qweight, qzeros, scales, g_idx) + bias.

Full shapes: x [8192, 4096] fp16, qweight [512, 4096] int32 (8x 4-bit packed
along K), qzeros [32, 512] int32, scales [32, 4096] fp16, g_idx [4096] int32
(k // 128), bias [4096] fp16.  Output [8192, 4096] fp16.

Strategy: 2 (M) x 4 (N) grid over 8 NeuronCores.  Per core: M=4096, N=1024,
K=4096, all in fp16 on the PE -- but via one level of Strassen, which cuts
PE cycles to 7/8 of the direct matmul (the PE at 78.6 TF/s fp16 is the
bottleneck; fp8 DoubleRow was measured to give 2x FLOPs per cycle but the
3-term error-corrected decomposition it needs costs 3x FLOPs, a net loss).

Per core split M=2x2048, K=2x2048, N=2x512:
  P1=(A11+A22)(B11+B22)  P2=(A21+A22)B11  P3=A11(B12-B22)  P4=A22(B21-B11)
  P5=(A11+A12)B22        P6=(A21-A11)(B11+B12)  P7=(A12-A22)(B21+B22)
  C11=P1+P4-P5+P7  C12=P3+P5  C21=P2+P4  C22=P1-P2+P3+P6

The host dequantizes W and precomputes the 7 fp16 A- and B-combos (adds of
quadrants; psum stays fp32 so the fp16 combo rounding is benign -- measured
rel err ~9e-4).  The device, per 128-row m'-block, accumulates the seven
P-products in seven PSUM banks (16 matmuls of [128k,128m]x[128k,512n] each)
and combines them with 12 DVE ops ordered so PSUM banks free early.  B-combos
(14.7MB fp16) stay resident in SBUF; A-combos stream per block on SyncE.
"""

import os
import sys

import numpy as np

for _p in ("/opt/trn_rl_repo",):
    if _p not in sys.path and os.path.isdir(_p):
        sys.path.insert(0, _p)

import concourse.bass as bass
import concourse.mybir as mybir
import concourse.tile as tile
from concourse import bacc
from concourse.bass_utils import run_bass_kernel_spmd

dt = mybir.dt

P = 128          # partitions
JP = 8           # 4-bit values per int32
GROUP = 128      # quant group size
NPS = 512        # psum free width / n'-quadrant width
NPROD = 7


def build_program(K, M, N):
    """One-core SPMD program: Strassen 1-level over [M=4096,K=4096,N=1024]."""
    KH, MH, NH = K // 2, M // 2, N // 2
    KT = KH // P         # 16 k'-tiles per product
    MB = MH // P         # 16 m'-blocks
    assert NH == NPS

    nc = bacc.Bacc("TRN2", target_bir_lowering=False)

    ac = nc.dram_tensor("ac", [NPROD, MB, P, KT, P], dt.float16, kind="ExternalInput")
    bc = nc.dram_tensor("bc", [NPROD, P, KT, NPS], dt.float16, kind="ExternalInput")
    bs = nc.dram_tensor("bs", [P, N], dt.float16, kind="ExternalInput")
    out = nc.dram_tensor("out", [M, N], dt.float16, kind="ExternalOutput")

    add = mybir.AluOpType.add
    sub = mybir.AluOpType.subtract

    from contextlib import ExitStack

    with tile.TileContext(nc) as tc, ExitStack() as ctx:
        const = ctx.enter_context(tc.tile_pool(name="const", bufs=1))
        bpool = ctx.enter_context(tc.tile_pool(name="bpool", bufs=2))
        apool = ctx.enter_context(tc.tile_pool(name="apool", bufs=12))
        cpool = ctx.enter_context(tc.tile_pool(name="cpool", bufs=18))
        opool = ctx.enter_context(tc.tile_pool(name="opool", bufs=6))
        psum = ctx.enter_context(tc.tile_pool(name="psum", bufs=8, space="PSUM"))

        # PE warmup: dummy fp16 matmuls with no DMA dependency so the HAM
        # clock-gate opens (1.2 -> 2.4 GHz) before the first real matmul.
        warm_src = const.tile([P, NPS], dt.float16)
        nc.gpsimd.memset(warm_src[:], 0.0)
        warm_ps = psum.tile([P, NPS], dt.float32, tag="ps")
        NWARM = 12
        for wi in range(NWARM):
            nc.tensor.matmul(
                warm_ps[:], warm_src[:, :P], warm_src[:],
                start=(wi == 0), stop=(wi == NWARM - 1),
            )

        bias_t = const.tile([P, N], dt.float16)
        nc.sync.dma_start(bias_t[:], bs[:])
        b0 = bias_t[:, 0:NPS]
        b1 = bias_t[:, NPS:N]

        # Global product-major sweep: for each product p, stream its A tiles
        # across all 16 m'-blocks against the single resident B[p] tile
        # (16KB/partition, double-buffered).  Startup only waits for
        # B[0]+A[0,0] (~2.6MB); B prefetch needs just 38GB/s.  Per-block
        # drains maintain 4 running fp16 accumulators via tensor_tensor ops
        # with ONE psum operand each; subtractions are ordered so the sign
        # works out with no negation pass, and bias rides the init op:
        #   C11 = P7-(P5-(P1+b0+P4));  C12 = (P3+b1)+P5;  C21 = (P2+b0)+P4
        #   C22 = P6+(P3-(P2-(P1+b1)))
        KTH = KT // 2
        a11 = [None] * MB; a12 = [None] * MB
        a21 = [None] * MB; a22 = [None] * MB

        def acc(tag, mb, in0, in1, op):
            t = cpool.tile([P, NPS], dt.float16, tag=tag,
                           name=f"{tag}_{mb}", bufs=18)
            nc.vector.tensor_tensor(t[:], in0, in1, op=op)
            return t

        def store(mb, u, nsl, upper):
            mrow = (mb * P) if upper else (MH + mb * P)
            nc.scalar.dma_start(out[mrow:mrow + P, nsl], u[:])

        for prod in range(NPROD):
            bt = bpool.tile([P, KT, NPS], dt.float16, tag="b", name=f"b{prod}")
            # quarter-DMAs alternating rings: subtile deps let the kt=0
            # matmuls start as soon as the first quarter lands (~3us)
            KTQ = KT // 4
            for q in range(4):
                qeng = nc.gpsimd if q % 2 == 0 else nc.scalar
                qeng.dma_start(bt[:, q * KTQ:(q + 1) * KTQ, :],
                               bc[prod, :, q * KTQ:(q + 1) * KTQ, :])
            for mb in range(MB):
                at = apool.tile([P, KT, P], dt.float16, tag="a",
                                name=f"a_t{prod}_{mb}", bufs=12)
                eng = nc.sync if mb % 2 == 0 else nc.gpsimd
                eng.dma_start(at[:], ac[prod, mb])
                ps = psum.tile([P, NPS], dt.float32, tag="ps",
                               name=f"ps{prod}_{mb}")
                for kt in range(KT):
                    nc.tensor.matmul(
                        ps[:], at[:, kt, :], bt[:, kt, :],
                        start=(kt == 0), stop=(kt == KT - 1),
                    )
                add = mybir.AluOpType.add
                sub = mybir.AluOpType.subtract
                if prod == 0:      # P1
                    a11[mb] = acc("c11", mb, ps[:], b0, add)
                    a22[mb] = acc("c22", mb, ps[:], b1, add)
                elif prod == 1:    # P2
                    a21[mb] = acc("c21", mb, ps[:], b0, add)
                    a22[mb] = acc("c22", mb, ps[:], a22[mb][:], sub)
                elif prod == 2:    # P3
                    a12[mb] = acc("c12", mb, ps[:], b1, add)
                    a22[mb] = acc("c22", mb, ps[:], a22[mb][:], sub)
                elif prod == 3:    # P4
                    a11[mb] = acc("c11", mb, ps[:], a11[mb][:], add)
                    o21 = opool.tile([P, NPS], dt.float16, tag="o",
                                     name="o21")
                    nc.vector.tensor_tensor(o21[:], ps[:], a21[mb][:], op=add)
                    store(mb, o21, slice(0, NPS), False)
                elif prod == 4:    # P5
                    a11[mb] = acc("c11", mb, ps[:], a11[mb][:], sub)
                    o12 = opool.tile([P, NPS], dt.float16, tag="o",
                                     name="o12")
                    nc.vector.tensor_tensor(o12[:], ps[:], a12[mb][:], op=add)
                    store(mb, o12, slice(NPS, N), True)
                elif prod == 5:    # P6
                    o22 = opool.tile([P, NPS], dt.float16, tag="o",
                                     name="o22")
                    nc.vector.tensor_tensor(o22[:], ps[:], a22[mb][:], op=add)
                    store(mb, o22, slice(NPS, N), False)
                else:              # P7
                    o11 = opool.tile([P, NPS], dt.float16, tag="o",
                                     name="o11")
                    nc.vector.tensor_tensor(o11[:], ps[:], a11[mb][:], op=sub)
                    store(mb, o11, slice(0, NPS), True)
    nc.finalize()
    return nc


def _pack_a(combo):
    """combo: [MH, KH] fp16 -> [MB, 128, KT, 128] with m=128*mb+mm, k=128*kt+p."""
    MH, KH = combo.shape
    a = combo.reshape(MH // P, P, KH // P, P)            # (mb, mm, kt, p)
    return np.ascontiguousarray(a.transpose(0, 3, 2, 1))  # (mb, p, kt, mm)


def _pack_b(combo):
    """combo: [KH, NPS] fp16 -> [128, KT, NPS] with k=128*kt+p."""
    KH = combo.shape[0]
    a = combo.reshape(KH // P, P, NPS)                   # (kt, p, n)
    return np.ascontiguousarray(a.transpose(1, 0, 2))    # (p, kt, n)


def host_prep(x, qweight, qzeros, scales, g_idx, bias, m_split, n_split):
    """Dequantize W, build fp16 Strassen A/B combos, pack per core."""
    M_full, K = x.shape
    G, N_full = scales.shape
    Mc = M_full // m_split
    Nc = N_full // n_split
    KH, MHc, NHc = K // 2, Mc // 2, Nc // 2

    shifts = (np.arange(JP, dtype=np.int32) * 4)
    w = ((qweight[:, None, :] >> shifts[None, :, None]) & 15).reshape(K, N_full)
    z = ((qzeros[:, :, None] >> shifts[None, None, :]) & 15).reshape(G, N_full) + 1
    cg = np.asarray(g_idx[::GROUP])
    assert np.array_equal(np.repeat(cg, GROUP), np.asarray(g_idx)), \
        "g_idx must be uniform within 128-wide k chunks"
    iw = (w - z[cg].repeat(GROUP, axis=0)).astype(np.float32)
    W16 = (iw * np.asarray(scales, np.float32)[cg].repeat(GROUP, axis=0)
           ).astype(np.float16)

    x = np.asarray(x)
    bias = np.asarray(bias)

    a_shards = []
    for mi in range(m_split):
        xm = x[mi * Mc:(mi + 1) * Mc]
        A11 = xm[:MHc, :KH]; A12 = xm[:MHc, KH:]
        A21 = xm[MHc:, :KH]; A22 = xm[MHc:, KH:]
        combos = (A11 + A22, A21 + A22, A11, A22,
                  A11 + A12, A21 - A11, A12 - A22)
        arr = np.empty((NPROD, MHc // P, P, KH // P, P), np.float16)
        for i, c in enumerate(combos):
            arr[i] = _pack_a(np.ascontiguousarray(c))
        a_shards.append(arr)

    b_shards = []
    for ni in range(n_split):
        Wc = W16[:, ni * Nc:(ni + 1) * Nc]
        B11 = Wc[:KH, :NHc]; B12 = Wc[:KH, NHc:]
        B21 = Wc[KH:, :NHc]; B22 = Wc[KH:, NHc:]
        combos = (B11 + B22, B11, B12 - B22, B21 - B11,
                  B22, B11 + B12, B21 + B22)
        arr = np.empty((NPROD, P, KH // P, NHc), np.float16)
        for i, c in enumerate(combos):
            arr[i] = _pack_b(np.ascontiguousarray(c))
        b_shards.append(arr)

    in_maps = []
    for mi in range(m_split):
        for ni in range(n_split):
            in_maps.append({
                "ac": a_shards[mi],
                "bc": b_shards[ni],
                "bs": np.ascontiguousarray(
                    np.broadcast_to(bias[ni * Nc:(ni + 1) * Nc], (P, Nc))
                ),
            })
    return in_maps, Mc, Nc


_PROGRAM_CACHE = {}


def _get_program(K, M, N):
    key = (K, M, N)
    if key not in _PROGRAM_CACHE:
        _PROGRAM_CACHE[key] = build_program(K, M, N)
    return _PROGRAM_CACHE[key]


def kernel(x, qweight, qzeros, scales, g_idx, bias, trace=False, trace_kwargs=None):
    m_split, n_split = 2, 4
    x = np.asarray(x)
    qweight = np.asarray(qweight)
    qzeros = np.asarray(qzeros)
    scales = np.asarray(scales)
    g_idx = np.asarray(g_idx)
    bias = np.asarray(bias)
    M_full, K = x.shape
    N_full = scales.shape[1]
    in_maps, M, N = host_prep(x, qweight, qzeros, scales, g_idx, bias,
                              m_split, n_split)
    nc = _get_program(K, M, N)
    kw = {}
    if trace:
        kw = dict(trace=True, **(trace_kwargs or {}))
    rb = run_bass_kernel_spmd(nc, in_maps, list(range(m_split * n_split)), **kw)
    out = np.empty((M_full, N_full), dtype=np.float16)
    ci = 0
    for mi in range(m_split):
        for ni in range(n_split):
            out[mi * M:(mi + 1) * M, ni * N:(ni + 1) * N] = rb.results[ci]["out"]
            ci += 1
    kernel.last_results = rb
    return out
